# revision 1
# baseline (speedup 1.0000x reference)
"""DKFN (graph-conv LSTM) Trainium2 kernel, V2.

Delta vs baseline:
- bf16 A-chain + bf16 phase-A AllGather payload (halves the big AG).
- G1/R1 SBUF-resident, computed just-in-time inside phase C so the
  GEMMs hide under the per-step AllGather stalls.
- Phase C: one fused AllGather per step (both paths' H^T in a single
  64KB-in non-shared collective) instead of two shared ones.
"""
import sys
import numpy as np
import ml_dtypes

sys.path.insert(0, '/opt/trn_rl_repo')
sys.path.insert(0, '/opt/trn_rl_repo/concourse')

import concourse.bass as bass
import concourse.mybir as mybir
import concourse.tile as tile
from concourse import bacc
from concourse.bass_utils import run_bass_kernel_spmd
from concourse.masks import make_identity

B, C, N, L, K, PRED = 64, 2, 1024, 24, 3, 12
R = B * C              # 128 rows (r = c*B + b, channel-major)
G = 8                  # cores
NS = N // G            # 128 nodes per core
JS = 4 * NS            # 512 gate features per core
LR = L * R             # 3072
P = 128
NCH = N // P           # 8 partition chunks of the node dim
KMC = K * NCH          # 24 chunks of the (k,m) dim
F32 = mybir.dt.float32
F32R = mybir.dt.float32r
BF16 = mybir.dt.bfloat16
F8 = mybir.dt.float8e4
ALU = mybir.AluOpType
ACTF = mybir.ActivationFunctionType


def build_program(reps=1, steps=None, skip_b=False, skip_ag=False,
                  skip_cag=False, shared_out=False):
    nc = bacc.Bacc("TRN2", target_bir_lowering=False, debug=False,
                   enable_asserts=False, num_devices=G)

    # ---- External inputs ----
    xsteps = nc.dram_tensor("xsteps", [L * P, NCH * R], BF16,
                            kind="ExternalInput").ap()
    adj = nc.dram_tensor("adj", [N, N], BF16, kind="ExternalInput").ap()
    adj_s = nc.dram_tensor("adj_s", [N, NS], BF16, kind="ExternalInput").ap()
    adjT_s = nc.dram_tensor("adjT_s", [N, NS], BF16, kind="ExternalInput").ap()
    gcw = nc.dram_tensor("gcw", [K * N, N], BF16, kind="ExternalInput").ap()
    gcwT_s = nc.dram_tensor("gcwT_s", [N, K * NS], BF16, kind="ExternalInput").ap()
    wgcT_s = nc.dram_tensor("wgcT_s", [K * N, JS], BF16, kind="ExternalInput").ap()
    wghT_s = nc.dram_tensor("wghT_s", [N, JS], BF16, kind="ExternalInput").ap()
    rwxT_s = nc.dram_tensor("rwxT_s", [N, JS], BF16, kind="ExternalInput").ap()
    rwghT_s = nc.dram_tensor("rwghT_s", [N, JS], BF16, kind="ExternalInput").ap()
    bg_s = nc.dram_tensor("bg_s", [JS], F32, kind="ExternalInput").ap()
    rbg_s = nc.dram_tensor("rbg_s", [JS], F32, kind="ExternalInput").ap()
    nw = nc.dram_tensor("nw", [N], F32, kind="ExternalInput").ap()

    # ---- External outputs ----
    H_out = nc.dram_tensor("H_out", [NS, R], F32, kind="ExternalOutput").ap()
    rH_out = nc.dram_tensor("rH_out", [NS, R], F32, kind="ExternalOutput").ap()
    v1part = nc.dram_tensor("v1part", [P, NCH * 4], F32, kind="ExternalOutput").ap()
    v2part = nc.dram_tensor("v2part", [P, K * 4], F32, kind="ExternalOutput").ap()

    xs_r = xsteps.rearrange("(l p) (c r) -> l p c r", p=P, r=R)
    adj_r = adj.rearrange("(c p) n -> p c n", p=P)
    adjs_r = adj_s.rearrange("(c p) m -> p c m", p=P)
    adjTs_r = adjT_s.rearrange("(c p) m -> p c m", p=P)
    gcw_r = gcw.rearrange("(c p) n -> p c n", p=P)
    gcwTs_r = gcwT_s.rearrange("(c p) j -> p c j", p=P)
    wgcTs_r = wgcT_s.rearrange("(c p) j -> p c j", p=P)
    wghTs_r = wghT_s.rearrange("(c p) j -> p c j", p=P)
    rwxTs_r = rwxT_s.rearrange("(c p) j -> p c j", p=P)
    rwghTs_r = rwghT_s.rearrange("(c p) j -> p c j", p=P)
    nw_r = nw.rearrange("(c p) -> p c", p=P)

    with tile.TileContext(nc) as tc:
      for _rep in range(reps):
        # ======================= Phase A: A-chain =======================
        const = tc.alloc_tile_pool(name="const", bufs=1, side="right")
        ident = const.tile([P, P], F32)
        make_identity(nc, ident)
        ones_col = const.tile([P, 1], BF16)
        nc.vector.memset(ones_col[:], 1.0)
        ones_row = const.tile([1, P], F32)
        nc.vector.memset(ones_row[:], 1.0)
        nbr_bc = const.tile([P, P], F32)
        bg_row = const.tile([1, JS], F32)
        rbg_row = const.tile([1, JS], F32)
        bg_bc = const.tile([P, JS], F32)
        rbg_bc = const.tile([P, JS], F32)
        nc.sync.dma_start(bg_row[:], bg_s.rearrange("(a j) -> a j", a=1))
        nc.sync.dma_start(rbg_row[:], rbg_s.rearrange("(a j) -> a j", a=1))

        dram = tc.alloc_tile_pool(name="dram", bufs=1, space="DRAM")
        pc_w = tc.alloc_tile_pool(name="pc_w", bufs=1, side="right")
        wgh_sb = pc_w.tile([P, NCH, JS], BF16)
        rwgh_sb = pc_w.tile([P, NCH, JS], BF16)
        nc.sync.dma_start(wgh_sb[:], wghTs_r)
        nc.sync.dma_start(rwgh_sb[:], rwghTs_r)

        paK = tc.alloc_tile_pool(name="paK", bufs=1)
        pa0 = tc.alloc_tile_pool(name="pa0", bufs=1)
        pa_ps = tc.alloc_tile_pool(name="pa_ps", bufs=2, space="PSUM")
        pa_ps1 = tc.alloc_tile_pool(name="pa_ps1", bufs=2, space="PSUM")

        # adj rows resident (bf16, 2MB) — used for tmp sums and anorm
        adj_sb = pa0.tile([P, NCH, N], BF16)
        nc.sync.dma_start(adj_sb[:], adj_r)

        tmp_row = pa0.tile([1, N], F32)
        ps_cs = [pa_ps1.tile([1, 512], F32, name=f"cs_ps{h}", tag=f"ps1_{h}")
                 for h in range(2)]
        for c_ in range(NCH):
            for h in range(2):
                nc.tensor.matmul(ps_cs[h][:], ones_col[:],
                                 adj_sb[:, c_, h * 512:(h + 1) * 512],
                                 start=(c_ == 0), stop=(c_ == NCH - 1))
        for h in range(2):
            nc.vector.tensor_scalar_max(tmp_row[:, h * 512:(h + 1) * 512],
                                        ps_cs[h][:], 1e-5)
        # roundtrip through DRAM to get tmp as per-partition columns (128, 8)
        tmp_dram = dram.tile([1, N], F32, name="tmp_dram")
        nc.sync.dma_start(tmp_dram[:], tmp_row[:])
        tmp_col = pa0.tile([P, NCH], F32)
        nc.sync.dma_start(tmp_col[:], tmp_dram[0, :].rearrange("(c p) -> p c", p=P))

        # a_norm[i, j] = tmp[i] * adj[i, j]  (bf16, SBUF-resident 2MB)
        anorm_sb = paK.tile([P, NCH, N], BF16)
        for c_ in range(NCH):
            nc.vector.tensor_scalar_mul(anorm_sb[:, c_], adj_sb[:, c_],
                                        tmp_col[:, c_:c_ + 1])

        # per-core column shard: tmp over this core's 128 columns
        adjs_sb = pa0.tile([P, NCH, NS], BF16)
        adjTs_sb = pa0.tile([P, NCH, NS], BF16)
        nc.scalar.dma_start(adjs_sb[:], adjs_r)
        nc.scalar.dma_start(adjTs_sb[:], adjTs_r)
        ps = pa_ps1.tile([1, NS], F32, name="tmps_ps", tag="ps1")
        for c_ in range(NCH):
            nc.tensor.matmul(ps[:], ones_col[:], adjs_sb[:, c_],
                             start=(c_ == 0), stop=(c_ == NCH - 1))
        tmps_row = pa0.tile([1, NS], F32)
        nc.vector.tensor_scalar_max(tmps_row[:], ps[:], 1e-5)
        ps = pa_ps.tile([P, NS], F32, name="tsb_ps", tag="ps")
        nc.tensor.matmul(ps[:], ones_row[:], tmps_row[:], start=True, stop=True)
        tsb = pa0.tile([P, NS], BF16)
        nc.vector.tensor_copy(tsb[:], ps[:])

        # B1_s[n, m] = min(adjT_s[n, m] * tmp_s[m], 1)   (column shard of A1^T)
        b1_sb = pa0.tile([P, NCH, NS], BF16)
        for c_ in range(NCH):
            nc.vector.tensor_tensor(b1_sb[:, c_], adjTs_sb[:, c_], tsb[:],
                                    op=ALU.mult)
            nc.vector.tensor_scalar_min(b1_sb[:, c_], b1_sb[:, c_], 1.0)

        # A2_s = rows of A2 = min(B1_s^T @ a_norm, 1); B-shards via PE
        # transposes of the row shards.
        b2_sb = pa0.tile([P, NCH, NS], BF16)
        b3_sb = pa0.tile([P, NCH, NS], BF16)
        a2s_sb = pa0.tile([P, NCH, P], F32)
        a3s_sb = pa0.tile([P, NCH, P], F32)
        for lhs_b, a_dst, b_dst in ((b1_sb, a2s_sb, b2_sb),
                                    (b2_sb, a3s_sb, b3_sb)):
            for h in range(2):
                ps = pa_ps.tile([P, 512], F32, name="chain_ps", tag="ps")
                for kc in range(NCH):
                    nc.tensor.matmul(
                        ps[:], lhs_b[:, kc],
                        anorm_sb[:, kc, h * 512:(h + 1) * 512],
                        start=(kc == 0), stop=(kc == NCH - 1))
                nc.vector.tensor_scalar_min(
                    a_dst[:, 4 * h:4 * (h + 1)].rearrange("p a b -> p (a b)"),
                    ps[:], 1.0)
            for c_ in range(NCH):
                ps = pa_ps.tile([P, P], F32, name="tr_ps", tag="ps")
                nc.tensor.transpose(ps[:], a_dst[:, c_], ident[:])
                nc.vector.tensor_copy(b_dst[:, c_], ps[:])

        # nbr for this core's nodes: nbr[m] = sum_n B3_s[n, m] * nw[n]
        nw_sb = pa0.tile([P, NCH], F32)
        nc.sync.dma_start(nw_sb[:], nw_r)
        nw_bf = pa0.tile([P, NCH], BF16)
        nc.vector.tensor_copy(nw_bf[:], nw_sb[:])
        ps = pa_ps.tile([P, 1], F32, name="nbr_ps", tag="ps")
        for c_ in range(NCH):
            nc.tensor.matmul(ps[:], b3_sb[:, c_], nw_bf[:, c_:c_ + 1],
                             start=(c_ == 0), stop=(c_ == NCH - 1))
        nbr_col = pa0.tile([P, 1], F32)
        nc.vector.tensor_copy(nbr_col[:], ps[:])
        # broadcast tiles for the (rows, features) gate layout
        nbr_dram = dram.tile([P, 1], F32, name="nbr_dram")
        nc.sync.dma_start(nbr_dram[:], nbr_col[:])
        nbr_row = pa0.tile([1, P], F32)
        nc.sync.dma_start(nbr_row[:], nbr_dram[:].rearrange("p a -> a p"))
        psb = pa_ps.tile([P, 512], F32, name="bc_ps", tag="ps")
        nc.tensor.matmul(psb[:, 0:P], ones_row[:], nbr_row[:], start=True, stop=True)
        nc.vector.tensor_copy(nbr_bc[:], psb[:, 0:P])
        nc.tensor.matmul(psb[:], ones_row[:], bg_row[:], start=True, stop=True)
        nc.vector.tensor_copy(bg_bc[:], psb[:])
        nc.tensor.matmul(psb[:], ones_row[:], rbg_row[:], start=True, stop=True)
        nc.vector.tensor_copy(rbg_bc[:], psb[:])

        # gc_last^T shard + variance partials (channel-major rows)
        gcwTs_sb = pa0.tile([P, NCH, K * P], BF16)
        nc.scalar.dma_start(gcwTs_sb[:], gcwTs_r)
        xlast_sb = pa0.tile([P, NCH, P], BF16)
        nc.scalar.dma_start(xlast_sb[:], xs_r[L - 1])
        effT_sb = pa0.tile([P, NCH, K * P], BF16)
        bks = (b1_sb, b2_sb, b3_sb)
        for c_ in range(NCH):
            for k_ in range(K):
                nc.vector.tensor_tensor(effT_sb[:, c_, k_ * P:(k_ + 1) * P],
                                        bks[k_][:, c_],
                                        gcwTs_sb[:, c_, k_ * P:(k_ + 1) * P],
                                        op=ALU.mult)
        v2_sb = pa0.tile([P, K, 4], F32)
        sq_sb = pa0.tile([P, B], F32)
        for k_ in range(K):
            ps = pa_ps.tile([P, P], F32, name="gcl_ps", tag="ps")
            for c_ in range(NCH):
                nc.tensor.matmul(ps[:], effT_sb[:, c_, k_ * P:(k_ + 1) * P],
                                 xlast_sb[:, c_],
                                 start=(c_ == 0), stop=(c_ == NCH - 1))
            gcl = pa0.tile([P, P], F32, name="gcl_sb", bufs=2)
            nc.vector.tensor_copy(gcl[:], ps[:])
            for ch in range(2):
                sl = slice(ch * B, (ch + 1) * B)
                nc.vector.tensor_reduce(v2_sb[:, k_, ch:ch + 1], gcl[:, sl],
                                        axis=mybir.AxisListType.X, op=ALU.add)
                nc.scalar.activation(sq_sb[:], gcl[:, sl], ACTF.Square,
                                     accum_out=v2_sb[:, k_, 2 + ch:3 + ch])
        nc.sync.dma_start(v2part[:], v2_sb[:].rearrange("p a b -> p (a b)"))
        v1_sb = pa0.tile([P, NCH, 4], F32)
        for c_ in range(NCH):
            for ch in range(2):
                sl = slice(ch * B, (ch + 1) * B)
                nc.vector.tensor_reduce(v1_sb[:, c_, ch:ch + 1],
                                        xlast_sb[:, c_, sl],
                                        axis=mybir.AxisListType.X, op=ALU.add)
                nc.scalar.activation(sq_sb[:], xlast_sb[:, c_, sl], ACTF.Square,
                                     accum_out=v1_sb[:, c_, 2 + ch:3 + ch])
        nc.sync.dma_start(v1part[:], v1_sb[:].rearrange("p a b -> p (a b)"))

        # AllGather row shards -> full A2 and A3 (bf16 payload)
        ag_a_in = dram.tile([2 * P, N], BF16, name="ag_a_in")
        ag_a_out = dram.tile([2 * N, N], BF16, name="ag_a_out",
                             **({} if skip_ag else dict(addr_space="Shared")))
        a23_bf = pa0.tile([P, 2, NCH, P], BF16)
        nc.vector.tensor_copy(a23_bf[:, 0], a2s_sb[:])
        nc.vector.tensor_copy(a23_bf[:, 1], a3s_sb[:])
        nc.sync.dma_start(
            ag_a_in[:].rearrange("(t p) n -> p t n", p=P),
            a23_bf[:].rearrange("p t c q -> p t (c q)"))
        if skip_ag:
            for d_ in range(G):
                nc.sync.dma_start(ag_a_out[d_ * 2 * P:(d_ + 1) * 2 * P, :],
                                  ag_a_in[:])
        else:
            nc.gpsimd.collective_compute(
                "AllGather", ALU.bypass, replica_groups=[list(range(G))],
                ins=[ag_a_in[:].opt()], outs=[ag_a_out[:].opt()])

        pa_ps1.release()
        pa_ps.release()
        pa0.release()

        # ================= Phase B: R1-all under the AG, then W_eff ========
        pb_w = tc.alloc_tile_pool(name="pb_w", bufs=1, side="right")
        pb_s = tc.alloc_tile_pool(name="pb_s", bufs=3)

        weff_sb = pb_w.tile([P, NCH, JS], BF16)      # 1 MB resident
        rwx_sb = pb_w.tile([P, NCH, JS], BF16)       # 1 MB resident
        nc.scalar.dma_start(rwx_sb[:], rwxTs_r)
        gcw12_sb = pb_w.tile([P, 2 * NCH, N], BF16)  # 4 MB resident (k=1,2)
        nc.scalar.dma_start(gcw12_sb[:], gcw_r[:, NCH:])

        pg1 = tc.alloc_tile_pool(name="pg1", bufs=1, side="right")
        g1_tiles = [pg1.tile([P, JS], BF16, name=f"g1_{l}", tag=f"g1_{l}")
                    for l in range(L)]
        r1_tiles = [pg1.tile([P, JS], BF16, name=f"r1_{l}", tag=f"r1_{l}")
                    for l in range(L)]
        pc_x = tc.alloc_tile_pool(name="pc_x", bufs=1)

        def load_xt(l, eng=None):
            xt = pc_x.tile([P, NCH, P], BF16, name="xt", tag=f"xt{l % 3}",
                           bufs=1)
            (eng or nc.scalar).dma_start(xt[:], xs_r[l])
            return xt

        # R1 for every step: independent of the A2/A3 AllGather, fills its
        # latency window.
        pb_pre = tc.alloc_tile_pool(name="pb_pre", bufs=2, space="PSUM")
        for l in range(L if not skip_b else 0):
            xt = load_xt(l, nc.sync)
            rp = pb_pre.tile([P, JS], F32, name="rp", tag="rp")
            for c_ in range(NCH):
                nc.tensor.matmul(rp[:], xt[:, c_], rwx_sb[:, c_],
                                 start=(c_ == 0), stop=(c_ == NCH - 1))
            nc.vector.tensor_tensor(r1_tiles[l][:], rp[:], rbg_bc[:], op=ALU.add)
        pb_pre.release()

        pb_ps = tc.alloc_tile_pool(name="pb_ps", bufs=1, space="PSUM")
        wps = [pb_ps.tile([P, JS], F32, name=f"wps{i}", tag=f"ps{i}")
               for i in range(NCH if not skip_b else 0)]
        ag_a_r = ag_a_out[:].rearrange("(d t p) n -> d t p n", t=2, p=P)
        for kmc in range(KMC if not skip_b else 0):
            k_, mc = divmod(kmc, NCH)
            eff_t = pb_s.tile([P, N], BF16, name="eff_t")
            if k_ == 0:
                gcw_t = pb_s.tile([P, N], BF16, name="gcw_t")
                nc.sync.dma_start(gcw_t[:], gcw_r[:, kmc])
                a1_t = pb_s.tile([P, N], BF16, name="a1_t")
                nc.vector.tensor_scalar_min(a1_t[:], anorm_sb[:, mc], 1.0)
                nc.vector.tensor_tensor(eff_t[:], a1_t[:], gcw_t[:],
                                        op=ALU.mult)
            else:
                a_t = pb_s.tile([P, N], BF16, name="a_t")
                nc.scalar.dma_start(a_t[:], ag_a_r[mc, k_ - 1])
                nc.vector.tensor_tensor(eff_t[:], a_t[:],
                                        gcw12_sb[:, kmc - NCH], op=ALU.mult)
            wgc_t = pb_s.tile([P, JS], BF16, name="wgc_t")
            nc.sync.dma_start(wgc_t[:], wgcTs_r[:, kmc])
            for ncc in range(NCH):
                nc.tensor.matmul(
                    wps[ncc][:],
                    eff_t[:, ncc * P:(ncc + 1) * P],
                    wgc_t[:],
                    start=(kmc == 0), stop=(kmc == KMC - 1))
        for ncc in range(NCH if not skip_b else 0):
            nc.vector.tensor_copy(weff_sb[:, ncc], wps[ncc][:])
        pb_ps.release()

        # ===================== Phase C: recurrence =====================
        # G1 computed just-in-time two steps ahead; one fused AllGather per
        # step carrying both paths' H^T shards.
        pc_st = tc.alloc_tile_pool(name="pc_st", bufs=2)
        pc_g = tc.alloc_tile_pool(name="pc_g", bufs=2)
        pc_ps = tc.alloc_tile_pool(name="pc_ps", bufs=1, space="PSUM")
        pc_psj = tc.alloc_tile_pool(name="pc_psj", bufs=1, space="PSUM")
        pc_pst = tc.alloc_tile_pool(name="pc_pst", bufs=1, space="PSUM")
        pc_dram = tc.alloc_tile_pool(name="pc_dram", bufs=2, space="DRAM")
        pc_shared = tc.alloc_tile_pool(name="pc_shared", bufs=2, space="DRAM")

        nsteps = L if steps is None else steps

        def jit_b(l, xt):
            """G1[l] from xt tile; g1_tiles[l] <- psum + bias."""
            gp = pc_psj.tile([P, JS], F32, name="gp", tag="gp")
            for c_ in range(NCH):
                nc.tensor.matmul(gp[:], xt[:, c_], weff_sb[:, c_],
                                 start=(c_ == 0), stop=(c_ == NCH - 1))
            nc.vector.tensor_tensor(g1_tiles[l][:], gp[:], bg_bc[:], op=ALU.add)

        xt_tiles = {}
        for l in range(min(3, nsteps + 2, L)):
            xt_tiles[l] = load_xt(l)
        for l in range(min(2, nsteps + 1, L) if not skip_b else 0):
            jit_b(l, xt_tiles.pop(l))

        cs_prev = pc_st.tile([P, P], F32, name="cs")
        csn_prev = pc_st.tile([P, P], F32, name="csn")
        rcs_prev = pc_st.tile([P, P], F32, name="rcs")
        nc.vector.memset(cs_prev[:], 0.0)
        nc.vector.memset(csn_prev[:], 0.0)
        nc.vector.memset(rcs_prev[:], 0.0)
        ht_prev = None
        rht_prev = None

        def halfstep(l, g1t, w_sb, state_prev, h_prev, use_nbr, out_ext, tag):
            """One path (g or r) of step l; returns (state, hT or None)."""
            last = (l == L - 1)
            if l > 0:
                pg = pc_ps.tile([P, JS], F32, name=f"pg{tag}", tag=f"pg{tag}")
                for c_ in range(NCH):
                    nc.tensor.matmul(pg[:], h_prev[:, c_], w_sb[:, c_],
                                     start=(c_ == 0), stop=(c_ == NCH - 1))
                pre = pc_g.tile([P, JS], F32, name=f"pre{tag}", bufs=1)
                nc.vector.tensor_tensor(pre[:], g1t[:], pg[:], op=ALU.add)
            else:
                pre = g1t
            sfio = pc_g.tile([P, 3 * P], F32, name=f"sfio{tag}", bufs=1)
            tct = pc_g.tile([P, P], F32, name=f"tct{tag}")
            nc.scalar.activation(sfio[:], pre[:, 0:3 * P], ACTF.Sigmoid)
            nc.scalar.activation(tct[:], pre[:, 3 * P:], ACTF.Tanh)
            sf, si, so = sfio[:, 0:P], sfio[:, P:2 * P], sfio[:, 2 * P:3 * P]
            u2 = pc_g.tile([P, P], F32, name=f"u2{tag}")
            mulsrc = state_prev[1] if use_nbr else state_prev
            nc.vector.tensor_tensor(u2[:], mulsrc[:], sf, op=ALU.mult)
            u3 = pc_g.tile([P, P], F32, name=f"u3{tag}")
            nc.vector.tensor_tensor(u3[:], si, tct[:], op=ALU.mult)
            state_new = pc_st.tile([P, P], F32, name=f"cs{tag}")
            nc.vector.tensor_tensor(state_new[:], u2[:], u3[:], op=ALU.add)
            tcs = pc_g.tile([P, P], F32, name=f"tcs{tag}")
            nc.scalar.activation(tcs[:], state_new[:], ACTF.Tanh)
            if use_nbr:
                csn_new = pc_st.tile([P, P], F32, name=f"csn{tag}")
                nc.vector.tensor_tensor(csn_new[:], state_new[:], nbr_bc[:],
                                        op=ALU.mult)
                state_ret = (state_new, csn_new)
            else:
                state_ret = state_new
            h_new = pc_g.tile([P, P], F32, name=f"hn{tag}")
            nc.vector.tensor_tensor(h_new[:], so, tcs[:], op=ALU.mult)
            # transpose (rows, nodes) -> (nodes, rows)
            pt = pc_pst.tile([P, P], F32, name=f"pt{tag}", tag=f"pt{tag}")
            nc.tensor.transpose(pt[:], h_new[:], ident[:])
            if last:
                hT = pc_g.tile([P, P], F32, name=f"hTf{tag}")
                nc.scalar.copy(hT[:], pt[:])
                nc.sync.dma_start(out_ext[:], hT[:])
                return state_ret, None
            return state_ret, pt

        gstate = (cs_prev, csn_prev)
        for l in range(nsteps):
            last = (l == L - 1)
            gstate, ptg = halfstep(l, g1_tiles[l], wgh_sb, gstate, ht_prev,
                                   True, H_out, "g")
            rcs_prev, ptr = halfstep(l, r1_tiles[l], rwgh_sb, rcs_prev,
                                     rht_prev, False, rH_out, "r")
            if not last:
                # pack both paths' H^T into one collective
                hpack = pc_g.tile([P, 2, P], F8, name="hpack")
                nc.vector.tensor_copy(hpack[:, 0], ptg[:])
                nc.vector.tensor_copy(hpack[:, 1], ptr[:])
                cc_in = pc_dram.tile([2 * P, P], F8, name="ccin")
                nc.sync.dma_start(
                    cc_in[:].rearrange("(t p) r -> p t r", p=P), hpack[:])
                shr = dict(addr_space="Shared") if shared_out else {}
                cc_out = pc_shared.tile([G * 2 * P, P], F8, name="ccout",
                                        tag="ccout", **shr)
                # JIT work fills the AllGather window
                if l + 3 < L and l + 3 < nsteps + 3:
                    xt_tiles[l + 3] = load_xt(l + 3)
                if l + 2 < L and not skip_b:
                    jit_b(l + 2, xt_tiles.pop(l + 2))
                if skip_cag:
                    for d_ in range(G):
                        nc.sync.dma_start(
                            cc_out[d_ * 2 * P:(d_ + 1) * 2 * P, :], cc_in[:])
                else:
                    nc.gpsimd.collective_compute(
                        "AllGather", ALU.bypass, replica_groups=[list(range(G))],
                        ins=[cc_in[:].opt()], outs=[cc_out[:].opt()])
                cc_r = cc_out[:].rearrange("(e t p) r -> p e t r", t=2, p=P)
                htg8 = pc_st.tile([P, NCH, P], F8, name="htg8", bufs=1)
                htr8 = pc_st.tile([P, NCH, P], F8, name="htr8", bufs=1)
                nc.sync.dma_start(htg8[:], cc_r[:, :, 0])
                nc.scalar.dma_start(htr8[:], cc_r[:, :, 1])
                ht_prev = pc_st.tile([P, NCH, P], BF16, name="htg")
                rht_prev = pc_st.tile([P, NCH, P], BF16, name="htr")
                nc.vector.tensor_copy(ht_prev[:], htg8[:])
                nc.vector.tensor_copy(rht_prev[:], htr8[:])

        for _pool in (pc_shared, pc_dram, pc_pst, pc_psj, pc_ps, pc_g, pc_st,
                      pc_x, pg1, pb_s, pb_w, paK, pc_w, dram, const):
            _pool.release()

    nc.compile()
    return nc


_CACHE = {}


def _get_nc():
    if 'nc' not in _CACHE:
        _CACHE['nc'] = build_program()
    return _CACHE['nc']


def _marshal(inputs):
    f = lambda a: np.ascontiguousarray(np.asarray(a, dtype=np.float32))
    bf = lambda a: np.ascontiguousarray(np.asarray(a)).astype(ml_dtypes.bfloat16)
    x = np.asarray(inputs['inputs'])                 # (B, C, N, L)
    xs = np.transpose(x, (3, 1, 0, 2)).reshape(LR, N)  # rows r = c*B + b
    # per-step, partition-contiguous layout: [l, p, c, r]
    xsteps = np.ascontiguousarray(
        xs.T.reshape(NCH, P, L, R).transpose(2, 1, 0, 3).reshape(L * P, NCH * R)
    ).astype(ml_dtypes.bfloat16)
    adj = np.asarray(inputs['adj'], dtype=np.float32)
    adjT = adj.T
    gcw = bf(np.asarray(inputs['gc_w']).reshape(K * N, N))
    gcwT = np.asarray(inputs['gc_w']).transpose(2, 0, 1).reshape(N, K * N)
    Wg = np.concatenate([inputs['Wf'], inputs['Wi'], inputs['Wo'], inputs['Wc']], 0)
    bg = np.concatenate([inputs['bf'], inputs['bi'], inputs['bo'], inputs['bc']], 0)
    rWg = np.concatenate([inputs['rWf'], inputs['rWi'], inputs['rWo'], inputs['rWc']], 0)
    rbg = np.concatenate([inputs['rbf'], inputs['rbi'], inputs['rbo'], inputs['rbc']], 0)
    in_maps = []
    for d in range(G):
        sl = slice(d * NS, (d + 1) * NS)
        jidx = np.concatenate([np.arange(g * N + d * NS, g * N + (d + 1) * NS)
                               for g in range(4)])
        Wg_rows = np.asarray(Wg)[jidx]
        rWg_rows = np.asarray(rWg)[jidx]
        in_maps.append({
            'xsteps': xsteps,
            'adj': bf(adj),
            'adj_s': bf(adj[:, sl]),
            'adjT_s': bf(adjT[:, sl]),
            'gcw': gcw,
            'gcwT_s': bf(gcwT[:, np.concatenate(
                [np.arange(k * N + d * NS, k * N + (d + 1) * NS)
                 for k in range(K)])]),
            'wgcT_s': np.ascontiguousarray(
                Wg_rows[:, :K * N].T.astype(ml_dtypes.bfloat16)),
            'wghT_s': np.ascontiguousarray(
                Wg_rows[:, K * N:].T.astype(ml_dtypes.bfloat16)),
            'rwxT_s': np.ascontiguousarray(
                rWg_rows[:, :N].T.astype(ml_dtypes.bfloat16)),
            'rwghT_s': np.ascontiguousarray(
                rWg_rows[:, N:].T.astype(ml_dtypes.bfloat16)),
            'bg_s': f(np.asarray(bg)[jidx]),
            'rbg_s': f(np.asarray(rbg)[jidx]),
            'nw': f(inputs['neighbor_w']),
        })
    return in_maps


def _assemble(results, inputs):
    H = np.zeros((R, N), np.float32)
    rH = np.zeros((R, N), np.float32)
    v2sum = np.zeros(2, np.float64)
    v2sq = np.zeros(2, np.float64)
    for d, res in enumerate(results):
        sl = slice(d * NS, (d + 1) * NS)
        H[:, sl] = res['H_out'].T
        rH[:, sl] = res['rH_out'].T
        v2p = res['v2part'].reshape(P, K, 4).astype(np.float64)
        v2sum += v2p[:, :, 0:2].sum((0, 1))
        v2sq += v2p[:, :, 2:4].sum((0, 1))
    v1p = results[0]['v1part'].reshape(P, NCH, 4).astype(np.float64)
    v1sum = v1p[:, :, 0:2].sum((0, 1))
    v1sq = v1p[:, :, 2:4].sum((0, 1))
    n1 = B * N
    n2 = B * K * N
    var1 = (v1sq - v1sum ** 2 / n1) / (n1 - 1)
    var2 = (v2sq - v2sum ** 2 / n2) / (n2 - 1)
    c = float(np.asarray(inputs['c'])[0])
    var1r = np.repeat(var1, B)[:, None].astype(np.float32)   # rows are c-major
    var2r = np.repeat(var2, B)[:, None].astype(np.float32)
    pred = (H * var1r * c + rH * var2r) / (var1r + var2r * c)  # (R, N)
    predB = pred.reshape(C, B, N).transpose(1, 0, 2)           # (B, C, N)
    conv_w = np.asarray(inputs['conv_w'], dtype=np.float32)    # (C*PRED, C)
    conv_b = np.asarray(inputs['conv_b'], dtype=np.float32)
    out = np.tensordot(predB, conv_w, axes=([1], [1]))         # (B, N, C*PRED)
    out = np.transpose(out, (0, 2, 1)) + conv_b[None, :, None]  # (B, C*PRED, N)
    return np.ascontiguousarray(out.reshape(B, C, N, PRED).astype(np.float32))


def kernel(**inputs):
    nc = _get_nc()
    in_maps = _marshal(inputs)
    res = run_bass_kernel_spmd(nc, in_maps, core_ids=list(range(G)))
    return _assemble(res.results, inputs)


if __name__ == "__main__":
    import time
    t0 = time.time()
    nc = _get_nc()
    print(f"build {time.time()-t0:.1f}s")



# revision 5
# speedup vs baseline: 1.3496x; 1.3496x over previous
"""DKFN (graph-conv LSTM) Trainium2 kernel, V3.

Delta vs V2 (536us):
- Per-step critical path shortened: fp8 DoubleRow H-GEMMs consume the
  AllGather payload directly (no bf16 converts), G1/R1 folded into the
  PSUM group via identity matmul, activations read PSUM.
- g/r elementwise packed into fused [P, 2*P] ops (half the DVE op count).
- R1 now JIT in phase C alongside G1 (fills AG windows, drops the
  phase-B R1 loop and the duplicate x pass).
- Phase-A AllGather kicked off right after the A-chain; nbr/gcl/variance
  work runs under it.
- Per-step collective output in Shared scratchpad.
"""
import sys
import numpy as np
import ml_dtypes

sys.path.insert(0, '/opt/trn_rl_repo')
sys.path.insert(0, '/opt/trn_rl_repo/concourse')

import concourse.bass as bass
import concourse.mybir as mybir
import concourse.tile as tile
from concourse import bacc
from concourse.bass_utils import run_bass_kernel_spmd
from concourse.masks import make_identity

B, C, N, L, K, PRED = 64, 2, 1024, 24, 3, 12
R = B * C              # 128 rows (r = c*B + b, channel-major)
G = 8                  # cores
NS = N // G            # 128 nodes per core
JS = 4 * NS            # 512 gate features per core
LR = L * R             # 3072
P = 128
NCH = N // P           # 8 partition chunks of the node dim
KMC = K * NCH          # 24 chunks of the (k,m) dim
F32 = mybir.dt.float32
F32R = mybir.dt.float32r
BF16 = mybir.dt.bfloat16
F8 = mybir.dt.float8e4
ALU = mybir.AluOpType
ACTF = mybir.ActivationFunctionType
DR = mybir.MatmulPerfMode.DoubleRow


def build_program(reps=1, steps=None, skip_b=False, skip_ag=False,
                  skip_cag=False, shared_out=True):
    nc = bacc.Bacc("TRN2", target_bir_lowering=False, debug=False,
                   enable_asserts=False, num_devices=G)

    # ---- External inputs ----
    xsteps = nc.dram_tensor("xsteps", [L * P, NCH * R], BF16,
                            kind="ExternalInput").ap()
    adj = nc.dram_tensor("adj", [N, N], BF16, kind="ExternalInput").ap()
    adj_s = nc.dram_tensor("adj_s", [N, NS], BF16, kind="ExternalInput").ap()
    adjT_s = nc.dram_tensor("adjT_s", [N, NS], BF16, kind="ExternalInput").ap()
    gcw = nc.dram_tensor("gcw", [K * N, N], BF16, kind="ExternalInput").ap()
    gcwT_s = nc.dram_tensor("gcwT_s", [N, K * NS], BF16, kind="ExternalInput").ap()
    wgcT_s = nc.dram_tensor("wgcT_s", [K * N, JS], BF16, kind="ExternalInput").ap()
    wghT_s = nc.dram_tensor("wghT_s", [N, JS], F8, kind="ExternalInput").ap()
    rwxT_s = nc.dram_tensor("rwxT_s", [N, JS], BF16, kind="ExternalInput").ap()
    rwghT_s = nc.dram_tensor("rwghT_s", [N, JS], F8, kind="ExternalInput").ap()
    bg_s = nc.dram_tensor("bg_s", [JS], F32, kind="ExternalInput").ap()
    rbg_s = nc.dram_tensor("rbg_s", [JS], F32, kind="ExternalInput").ap()
    nw = nc.dram_tensor("nw", [N], F32, kind="ExternalInput").ap()

    # ---- External outputs ----
    H_out = nc.dram_tensor("H_out", [NS, R], F32, kind="ExternalOutput").ap()
    rH_out = nc.dram_tensor("rH_out", [NS, R], F32, kind="ExternalOutput").ap()
    v1part = nc.dram_tensor("v1part", [P, NCH * 4], F32, kind="ExternalOutput").ap()
    v2part = nc.dram_tensor("v2part", [P, K * 4], F32, kind="ExternalOutput").ap()

    xs_r = xsteps.rearrange("(l p) (c r) -> l p c r", p=P, r=R)
    adj_r = adj.rearrange("(c p) n -> p c n", p=P)
    adjs_r = adj_s.rearrange("(c p) m -> p c m", p=P)
    adjTs_r = adjT_s.rearrange("(c p) m -> p c m", p=P)
    gcw_r = gcw.rearrange("(c p) n -> p c n", p=P)
    gcwTs_r = gcwT_s.rearrange("(c p) j -> p c j", p=P)
    wgcTs_r = wgcT_s.rearrange("(c p) j -> p c j", p=P)
    wghTs_r = wghT_s.rearrange("(c p) j -> p c j", p=P)
    rwxTs_r = rwxT_s.rearrange("(c p) j -> p c j", p=P)
    rwghTs_r = rwghT_s.rearrange("(c p) j -> p c j", p=P)
    nw_r = nw.rearrange("(c p) -> p c", p=P)

    with tile.TileContext(nc) as tc:
      for _rep in range(reps):
        # ======================= Phase A: A-chain =======================
        const = tc.alloc_tile_pool(name="const", bufs=1, side="right")
        ident = const.tile([P, P], F32)
        make_identity(nc, ident)
        identb = const.tile([P, P], BF16)
        nc.vector.tensor_copy(identb[:], ident[:])
        ones_col = const.tile([P, 1], BF16)
        nc.vector.memset(ones_col[:], 1.0)
        ones_row = const.tile([1, P], F32)
        nc.vector.memset(ones_row[:], 1.0)
        nbr_bc = const.tile([P, P], F32)
        bg_row = const.tile([1, JS], F32)
        rbg_row = const.tile([1, JS], F32)
        bg_bc = const.tile([P, JS], F32)
        rbg_bc = const.tile([P, JS], F32)
        nc.sync.dma_start(bg_row[:], bg_s.rearrange("(a j) -> a j", a=1))
        nc.sync.dma_start(rbg_row[:], rbg_s.rearrange("(a j) -> a j", a=1))

        dram = tc.alloc_tile_pool(name="dram", bufs=1, space="DRAM")
        pc_w = tc.alloc_tile_pool(name="pc_w", bufs=1, side="right")
        wgh_sb = pc_w.tile([P, NCH, JS], F8)
        rwgh_sb = pc_w.tile([P, NCH, JS], F8)
        nc.sync.dma_start(wgh_sb[:], wghTs_r)
        nc.sync.dma_start(rwgh_sb[:], rwghTs_r)

        paK = tc.alloc_tile_pool(name="paK", bufs=1)
        pa0 = tc.alloc_tile_pool(name="pa0", bufs=1)
        pa_ps = tc.alloc_tile_pool(name="pa_ps", bufs=2, space="PSUM")
        pa_ps1 = tc.alloc_tile_pool(name="pa_ps1", bufs=2, space="PSUM")

        # adj rows resident (bf16, 2MB) — used for tmp sums and anorm
        adj_sb = pa0.tile([P, NCH, N], BF16)
        nc.sync.dma_start(adj_sb[:], adj_r)

        tmp_row = pa0.tile([1, N], F32)
        ps_cs = [pa_ps1.tile([1, 512], F32, name=f"cs_ps{h}", tag=f"ps1_{h}")
                 for h in range(2)]
        for c_ in range(NCH):
            for h in range(2):
                nc.tensor.matmul(ps_cs[h][:], ones_col[:],
                                 adj_sb[:, c_, h * 512:(h + 1) * 512],
                                 start=(c_ == 0), stop=(c_ == NCH - 1))
        for h in range(2):
            nc.vector.tensor_scalar_max(tmp_row[:, h * 512:(h + 1) * 512],
                                        ps_cs[h][:], 1e-5)
        # roundtrip through DRAM to get tmp as per-partition columns (128, 8)
        tmp_dram = dram.tile([1, N], F32, name="tmp_dram")
        nc.sync.dma_start(tmp_dram[:], tmp_row[:])
        tmp_col = pa0.tile([P, NCH], F32)
        nc.sync.dma_start(tmp_col[:], tmp_dram[0, :].rearrange("(c p) -> p c", p=P))

        # a_norm[i, j] = tmp[i] * adj[i, j]  (bf16, SBUF-resident 2MB)
        anorm_sb = paK.tile([P, NCH, N], BF16)
        for c_ in range(NCH):
            nc.vector.tensor_scalar_mul(anorm_sb[:, c_], adj_sb[:, c_],
                                        tmp_col[:, c_:c_ + 1])

        # per-core column shard: tmp over this core's 128 columns
        adjs_sb = pa0.tile([P, NCH, NS], BF16)
        adjTs_sb = pa0.tile([P, NCH, NS], BF16)
        nc.scalar.dma_start(adjs_sb[:], adjs_r)
        nc.scalar.dma_start(adjTs_sb[:], adjTs_r)
        ps = pa_ps1.tile([1, NS], F32, name="tmps_ps", tag="ps1")
        for c_ in range(NCH):
            nc.tensor.matmul(ps[:], ones_col[:], adjs_sb[:, c_],
                             start=(c_ == 0), stop=(c_ == NCH - 1))
        tmps_row = pa0.tile([1, NS], F32)
        nc.vector.tensor_scalar_max(tmps_row[:], ps[:], 1e-5)
        ps = pa_ps.tile([P, NS], F32, name="tsb_ps", tag="ps")
        nc.tensor.matmul(ps[:], ones_row[:], tmps_row[:], start=True, stop=True)
        tsb = pa0.tile([P, NS], BF16)
        nc.vector.tensor_copy(tsb[:], ps[:])

        # B1_s[n, m] = min(adjT_s[n, m] * tmp_s[m], 1)   (column shard of A1^T)
        b1_sb = pa0.tile([P, NCH, NS], BF16)
        for c_ in range(NCH):
            nc.vector.tensor_tensor(b1_sb[:, c_], adjTs_sb[:, c_], tsb[:],
                                    op=ALU.mult)
            nc.vector.tensor_scalar_min(b1_sb[:, c_], b1_sb[:, c_], 1.0)

        # A2_s = rows of A2 = min(B1_s^T @ a_norm, 1); B-shards via PE
        # transposes of the row shards.
        b2_sb = pa0.tile([P, NCH, NS], BF16)
        b3_sb = pa0.tile([P, NCH, NS], BF16)
        a2s_sb = pa0.tile([P, NCH, P], F32)
        a3s_sb = pa0.tile([P, NCH, P], F32)
        for lhs_b, a_dst, b_dst in ((b1_sb, a2s_sb, b2_sb),
                                    (b2_sb, a3s_sb, b3_sb)):
            for h in range(2):
                ps = pa_ps.tile([P, 512], F32, name="chain_ps", tag="ps")
                for kc in range(NCH):
                    nc.tensor.matmul(
                        ps[:], lhs_b[:, kc],
                        anorm_sb[:, kc, h * 512:(h + 1) * 512],
                        start=(kc == 0), stop=(kc == NCH - 1))
                nc.vector.tensor_scalar_min(
                    a_dst[:, 4 * h:4 * (h + 1)].rearrange("p a b -> p (a b)"),
                    ps[:], 1.0)
            for c_ in range(NCH):
                ps = pa_ps.tile([P, P], F32, name="tr_ps", tag="ps")
                nc.tensor.transpose(ps[:], a_dst[:, c_], ident[:])
                nc.vector.tensor_copy(b_dst[:, c_], ps[:])

        # AllGather row shards -> full A2 and A3 (bf16 payload); kicked off
        # as early as possible, everything below overlaps its latency.
        ag_a_in = dram.tile([2 * P, N], BF16, name="ag_a_in")
        ag_a_out = dram.tile([2 * N, N], BF16, name="ag_a_out",
                             **({} if skip_ag else dict(addr_space="Shared")))
        a23_bf = pa0.tile([P, 2, NCH, P], BF16)
        nc.vector.tensor_copy(a23_bf[:, 0], a2s_sb[:])
        nc.vector.tensor_copy(a23_bf[:, 1], a3s_sb[:])
        nc.sync.dma_start(
            ag_a_in[:].rearrange("(t p) n -> p t n", p=P),
            a23_bf[:].rearrange("p t c q -> p t (c q)"))
        if skip_ag:
            for d_ in range(G):
                nc.sync.dma_start(ag_a_out[d_ * 2 * P:(d_ + 1) * 2 * P, :],
                                  ag_a_in[:])
        else:
            nc.gpsimd.collective_compute(
                "AllGather", ALU.bypass, replica_groups=[list(range(G))],
                ins=[ag_a_in[:].opt()], outs=[ag_a_out[:].opt()])

        # nbr for this core's nodes: nbr[m] = sum_n B3_s[n, m] * nw[n]
        nw_sb = pa0.tile([P, NCH], F32)
        nc.sync.dma_start(nw_sb[:], nw_r)
        nw_bf = pa0.tile([P, NCH], BF16)
        nc.vector.tensor_copy(nw_bf[:], nw_sb[:])
        ps = pa_ps.tile([P, 1], F32, name="nbr_ps", tag="ps")
        for c_ in range(NCH):
            nc.tensor.matmul(ps[:], b3_sb[:, c_], nw_bf[:, c_:c_ + 1],
                             start=(c_ == 0), stop=(c_ == NCH - 1))
        nbr_col = pa0.tile([P, 1], F32)
        nc.vector.tensor_copy(nbr_col[:], ps[:])
        # broadcast nbr over rows: (rows, nodes) tile, constant down columns
        nbr_dram = dram.tile([P, 1], F32, name="nbr_dram")
        nc.sync.dma_start(nbr_dram[:], nbr_col[:])
        nbr_row = pa0.tile([1, P], F32)
        nc.sync.dma_start(nbr_row[:], nbr_dram[:].rearrange("p a -> a p"))
        psb = pa_ps.tile([P, 512], F32, name="bc_ps", tag="ps")
        nc.tensor.matmul(psb[:, 0:P], ones_row[:], nbr_row[:], start=True,
                         stop=True)
        nc.vector.tensor_copy(nbr_bc[:], psb[:, 0:P])
        nc.tensor.matmul(psb[:], ones_row[:], bg_row[:], start=True, stop=True)
        nc.vector.tensor_copy(bg_bc[:], psb[:])
        nc.tensor.matmul(psb[:], ones_row[:], rbg_row[:], start=True, stop=True)
        nc.vector.tensor_copy(rbg_bc[:], psb[:])

        # gc_last^T shard + variance partials (channel-major rows)
        gcwTs_sb = pa0.tile([P, NCH, K * P], BF16)
        nc.scalar.dma_start(gcwTs_sb[:], gcwTs_r)
        xlast_sb = pa0.tile([P, NCH, P], BF16)
        nc.scalar.dma_start(xlast_sb[:], xs_r[L - 1])
        effT_sb = pa0.tile([P, NCH, K * P], BF16)
        bks = (b1_sb, b2_sb, b3_sb)
        for c_ in range(NCH):
            for k_ in range(K):
                nc.vector.tensor_tensor(effT_sb[:, c_, k_ * P:(k_ + 1) * P],
                                        bks[k_][:, c_],
                                        gcwTs_sb[:, c_, k_ * P:(k_ + 1) * P],
                                        op=ALU.mult)
        v2_sb = pa0.tile([P, K, 4], F32)
        sq_sb = pa0.tile([P, B], F32)
        for k_ in range(K):
            ps = pa_ps.tile([P, P], F32, name="gcl_ps", tag="ps")
            for c_ in range(NCH):
                nc.tensor.matmul(ps[:], effT_sb[:, c_, k_ * P:(k_ + 1) * P],
                                 xlast_sb[:, c_],
                                 start=(c_ == 0), stop=(c_ == NCH - 1))
            gcl = pa0.tile([P, P], F32, name="gcl_sb", bufs=2)
            nc.vector.tensor_copy(gcl[:], ps[:])
            for ch in range(2):
                sl = slice(ch * B, (ch + 1) * B)
                nc.vector.tensor_reduce(v2_sb[:, k_, ch:ch + 1], gcl[:, sl],
                                        axis=mybir.AxisListType.X, op=ALU.add)
                nc.scalar.activation(sq_sb[:], gcl[:, sl], ACTF.Square,
                                     accum_out=v2_sb[:, k_, 2 + ch:3 + ch])
        nc.sync.dma_start(v2part[:], v2_sb[:].rearrange("p a b -> p (a b)"))
        v1_sb = pa0.tile([P, NCH, 4], F32)
        for c_ in range(NCH):
            for ch in range(2):
                sl = slice(ch * B, (ch + 1) * B)
                nc.vector.tensor_reduce(v1_sb[:, c_, ch:ch + 1],
                                        xlast_sb[:, c_, sl],
                                        axis=mybir.AxisListType.X, op=ALU.add)
                nc.scalar.activation(sq_sb[:], xlast_sb[:, c_, sl], ACTF.Square,
                                     accum_out=v1_sb[:, c_, 2 + ch:3 + ch])
        nc.sync.dma_start(v1part[:], v1_sb[:].rearrange("p a b -> p (a b)"))

        pa_ps1.release()
        pa_ps.release()
        pa0.release()

        # ================= Phase B: W_eff build ========
        pb_w = tc.alloc_tile_pool(name="pb_w", bufs=1, side="right")
        pb_s = tc.alloc_tile_pool(name="pb_s", bufs=3)

        weff_sb = pb_w.tile([P, NCH, JS], BF16)      # 1 MB resident
        rwx_sb = pb_w.tile([P, NCH, JS], BF16)       # 1 MB resident
        nc.scalar.dma_start(rwx_sb[:], rwxTs_r)
        gcw12_sb = pb_w.tile([P, 2 * NCH, N], BF16)  # 4 MB resident (k=1,2)
        nc.scalar.dma_start(gcw12_sb[:], gcw_r[:, NCH:])

        pg1 = tc.alloc_tile_pool(name="pg1", bufs=1, side="right")
        g1_tiles = [pg1.tile([P, JS], BF16, name=f"g1_{l}", tag=f"g1_{l}")
                    for l in range(L)]
        r1_tiles = [pg1.tile([P, JS], BF16, name=f"r1_{l}", tag=f"r1_{l}")
                    for l in range(L)]
        if skip_b:
            for l in range(L):
                nc.vector.memset(g1_tiles[l][:], 0.0)
                nc.vector.memset(r1_tiles[l][:], 0.0)
        pc_x = tc.alloc_tile_pool(name="pc_x", bufs=1)

        def load_xt(l, eng=None):
            xt = pc_x.tile([P, NCH, P], BF16, name="xt", tag=f"xt{l % 3}",
                           bufs=1)
            (eng or nc.scalar).dma_start(xt[:], xs_r[l])
            return xt

        pb_ps = tc.alloc_tile_pool(name="pb_ps", bufs=1, space="PSUM")
        wps = [pb_ps.tile([P, JS], F32, name=f"wps{i}", tag=f"ps{i}")
               for i in range(NCH if not skip_b else 0)]
        ag_a_r = ag_a_out[:].rearrange("(d t p) n -> d t p n", t=2, p=P)
        for kmc in range(KMC if not skip_b else 0):
            k_, mc = divmod(kmc, NCH)
            eff_t = pb_s.tile([P, N], BF16, name="eff_t")
            if k_ == 0:
                gcw_t = pb_s.tile([P, N], BF16, name="gcw_t")
                nc.sync.dma_start(gcw_t[:], gcw_r[:, kmc])
                a1_t = pb_s.tile([P, N], BF16, name="a1_t")
                nc.vector.tensor_scalar_min(a1_t[:], anorm_sb[:, mc], 1.0)
                nc.vector.tensor_tensor(eff_t[:], a1_t[:], gcw_t[:],
                                        op=ALU.mult)
            else:
                a_t = pb_s.tile([P, N], BF16, name="a_t")
                nc.scalar.dma_start(a_t[:], ag_a_r[mc, k_ - 1])
                nc.vector.tensor_tensor(eff_t[:], a_t[:],
                                        gcw12_sb[:, kmc - NCH], op=ALU.mult)
            wgc_t = pb_s.tile([P, JS], BF16, name="wgc_t")
            nc.sync.dma_start(wgc_t[:], wgcTs_r[:, kmc])
            for ncc in range(NCH):
                nc.tensor.matmul(
                    wps[ncc][:],
                    eff_t[:, ncc * P:(ncc + 1) * P],
                    wgc_t[:],
                    start=(kmc == 0), stop=(kmc == KMC - 1))
        for ncc in range(NCH if not skip_b else 0):
            nc.vector.tensor_copy(weff_sb[:, ncc], wps[ncc][:])
        pb_ps.release()

        # ===================== Phase C: recurrence =====================
        # G1+R1 computed just-in-time two steps ahead (fills AllGather
        # windows); one fused AllGather per step with both paths' H^T.
        pc_st = tc.alloc_tile_pool(name="pc_st", bufs=2)
        pc_g = tc.alloc_tile_pool(name="pc_g", bufs=2)
        pc_ps = tc.alloc_tile_pool(name="pc_ps", bufs=1, space="PSUM")
        pc_psj = tc.alloc_tile_pool(name="pc_psj", bufs=1, space="PSUM")
        pc_pst = tc.alloc_tile_pool(name="pc_pst", bufs=1, space="PSUM")
        pc_dram = tc.alloc_tile_pool(name="pc_dram", bufs=2, space="DRAM")
        pc_shared = tc.alloc_tile_pool(name="pc_shared", bufs=2, space="DRAM")

        nsteps = L if steps is None else steps

        def jit_b(l, xt):
            """G1[l], R1[l] from xt tile (bias added, bf16 SBUF)."""
            gp = pc_psj.tile([P, JS], F32, name="gp", tag="gp")
            for c_ in range(NCH):
                nc.tensor.matmul(gp[:], xt[:, c_], weff_sb[:, c_],
                                 start=(c_ == 0), stop=(c_ == NCH - 1))
            nc.vector.tensor_tensor(g1_tiles[l][:], gp[:], bg_bc[:], op=ALU.add)
            rp = pc_psj.tile([P, JS], F32, name="rp", tag="rp")
            for c_ in range(NCH):
                nc.tensor.matmul(rp[:], xt[:, c_], rwx_sb[:, c_],
                                 start=(c_ == 0), stop=(c_ == NCH - 1))
            nc.vector.tensor_tensor(r1_tiles[l][:], rp[:], rbg_bc[:], op=ALU.add)

        xt_tiles = {}
        for l in range(min(3, nsteps + 2, L)):
            xt_tiles[l] = load_xt(l)
        for l in range(min(2, nsteps + 1, L) if not skip_b else 0):
            jit_b(l, xt_tiles.pop(l))

        # packed states: [:, 0, :] = g path, [:, 1, :] = r path
        cs_prev = pc_st.tile([P, 2, P], F32, name="cs")
        mul_prev = pc_st.tile([P, 2, P], F32, name="muls")
        nc.vector.memset(cs_prev[:], 0.0)
        nc.vector.memset(mul_prev[:], 0.0)
        ht8_prev = None

        for l in range(nsteps):
            last = (l == L - 1)
            # --- gate pre-activations for both paths ---
            pres = []
            for t, (g1t, w_sb, tagp) in enumerate(
                    ((g1_tiles[l], wgh_sb, "g"), (r1_tiles[l], rwgh_sb, "r"))):
                if l > 0:
                    pg = pc_ps.tile([P, JS], F32, name=f"pg{tagp}",
                                    tag=f"pg{tagp}")
                    for c_ in range(0, NCH, 2):
                        nc.tensor.matmul(pg[:], ht8_prev[:, t, c_:c_ + 2],
                                         w_sb[:, c_:c_ + 2],
                                         start=(c_ == 0), stop=False,
                                         perf_mode=DR)
                    nc.tensor.matmul(pg[:], identb[:], g1t[:],
                                     start=False, stop=True)
                    pres.append(pg)
                else:
                    pres.append(g1t)
            # --- packed activations: sfio [P, 2, 3P], tct [P, 2, P] ---
            sfio = pc_g.tile([P, 2, 3 * P], F32, name="sfio", bufs=1)
            tct = pc_g.tile([P, 2, P], F32, name="tct")
            for t in range(2):
                nc.scalar.activation(sfio[:, t], pres[t][:, 0:3 * P],
                                     ACTF.Sigmoid)
                nc.scalar.activation(tct[:, t], pres[t][:, 3 * P:], ACTF.Tanh)
            sf = sfio[:, :, 0:P]
            si = sfio[:, :, P:2 * P]
            so = sfio[:, :, 2 * P:3 * P]
            # --- packed cell update ---
            u2 = pc_g.tile([P, 2, P], F32, name="u2")
            nc.vector.tensor_tensor(u2[:], mul_prev[:], sf, op=ALU.mult)
            u3 = pc_g.tile([P, 2, P], F32, name="u3")
            nc.vector.tensor_tensor(u3[:], si, tct[:], op=ALU.mult)
            cs_new = pc_st.tile([P, 2, P], F32, name="cs")
            nc.vector.tensor_tensor(cs_new[:], u2[:], u3[:], op=ALU.add)
            tcs = pc_g.tile([P, 2, P], F32, name="tcs")
            nc.scalar.activation(tcs[:], cs_new[:], ACTF.Tanh)
            # mul_prev for next step: g half gets nbr scaling, r half copies
            mul_new = pc_st.tile([P, 2, P], F32, name="muls")
            nc.vector.tensor_tensor(mul_new[:, 0], cs_new[:, 0], nbr_bc[:],
                                    op=ALU.mult)
            nc.vector.tensor_copy(mul_new[:, 1], cs_new[:, 1])
            h_new = pc_g.tile([P, 2, P], F32, name="hn")
            nc.vector.tensor_tensor(h_new[:], so, tcs[:], op=ALU.mult)
            cs_prev, mul_prev = cs_new, mul_new
            # transpose (rows, nodes) -> (nodes, rows) per path
            ptg = pc_pst.tile([P, P], F32, name="ptg", tag="ptg")
            nc.tensor.transpose(ptg[:], h_new[:, 0], ident[:])
            ptr = pc_pst.tile([P, P], F32, name="ptr", tag="ptr")
            nc.tensor.transpose(ptr[:], h_new[:, 1], ident[:])
            if last:
                hTg = pc_g.tile([P, P], F32, name="hTfg")
                nc.scalar.copy(hTg[:], ptg[:])
                nc.sync.dma_start(H_out[:], hTg[:])
                hTr = pc_g.tile([P, P], F32, name="hTfr")
                nc.scalar.copy(hTr[:], ptr[:])
                nc.sync.dma_start(rH_out[:], hTr[:])
                continue
            # pack both paths' H^T into one collective (fp8)
            hpack = pc_g.tile([P, 2, P], F8, name="hpack")
            nc.vector.tensor_copy(hpack[:, 0], ptg[:])
            nc.vector.tensor_copy(hpack[:, 1], ptr[:])
            cc_in = pc_dram.tile([2 * P, P], F8, name="ccin")
            nc.sync.dma_start(
                cc_in[:].rearrange("(t p) r -> p t r", p=P), hpack[:])
            shr = dict(addr_space="Shared") if (shared_out and not skip_cag) else {}
            cc_out = pc_shared.tile([G * 2 * P, P], F8, name="ccout",
                                    tag="ccout", **shr)
            # JIT work fills the AllGather window
            if l + 3 < L and l + 3 < nsteps + 3:
                xt_tiles[l + 3] = load_xt(l + 3)
            if l + 2 < L and not skip_b:
                jit_b(l + 2, xt_tiles.pop(l + 2))
            if skip_cag:
                for d_ in range(G):
                    nc.sync.dma_start(
                        cc_out[d_ * 2 * P:(d_ + 1) * 2 * P, :], cc_in[:])
            else:
                nc.gpsimd.collective_compute(
                    "AllGather", ALU.bypass, replica_groups=[list(range(G))],
                    ins=[cc_in[:].opt()], outs=[cc_out[:].opt()])
            # gathered H^T, path-major: ht8[p, t, e, r]
            cc_r = cc_out[:].rearrange("(e t p) r -> p t e r", t=2, p=P)
            ht8 = pc_st.tile([P, 2, NCH, P], F8, name="ht8")
            nc.sync.dma_start(ht8[:, 0], cc_r[:, 0])
            nc.scalar.dma_start(ht8[:, 1], cc_r[:, 1])
            ht8_prev = ht8

        for _pool in (pc_shared, pc_dram, pc_pst, pc_psj, pc_ps, pc_g, pc_st,
                      pc_x, pg1, pb_s, pb_w, paK, pc_w, dram, const):
            _pool.release()

    nc.compile()
    return nc


_CACHE = {}


def _get_nc():
    if 'nc' not in _CACHE:
        _CACHE['nc'] = build_program()
    return _CACHE['nc']


def _marshal(inputs):
    f = lambda a: np.ascontiguousarray(np.asarray(a, dtype=np.float32))
    bf = lambda a: np.ascontiguousarray(np.asarray(a)).astype(ml_dtypes.bfloat16)
    f8 = lambda a: np.ascontiguousarray(np.asarray(a)).astype(ml_dtypes.float8_e4m3)
    x = np.asarray(inputs['inputs'])                 # (B, C, N, L)
    xs = np.transpose(x, (3, 1, 0, 2)).reshape(LR, N)  # rows r = c*B + b
    # per-step, partition-contiguous layout: [l, p, c, r]
    xsteps = np.ascontiguousarray(
        xs.T.reshape(NCH, P, L, R).transpose(2, 1, 0, 3).reshape(L * P, NCH * R)
    ).astype(ml_dtypes.bfloat16)
    adj = np.asarray(inputs['adj'], dtype=np.float32)
    adjT = adj.T
    gcw = bf(np.asarray(inputs['gc_w']).reshape(K * N, N))
    gcwT = np.asarray(inputs['gc_w']).transpose(2, 0, 1).reshape(N, K * N)
    Wg = np.concatenate([inputs['Wf'], inputs['Wi'], inputs['Wo'], inputs['Wc']], 0)
    bg = np.concatenate([inputs['bf'], inputs['bi'], inputs['bo'], inputs['bc']], 0)
    rWg = np.concatenate([inputs['rWf'], inputs['rWi'], inputs['rWo'], inputs['rWc']], 0)
    rbg = np.concatenate([inputs['rbf'], inputs['rbi'], inputs['rbo'], inputs['rbc']], 0)
    in_maps = []
    for d in range(G):
        sl = slice(d * NS, (d + 1) * NS)
        jidx = np.concatenate([np.arange(g * N + d * NS, g * N + (d + 1) * NS)
                               for g in range(4)])
        Wg_rows = np.asarray(Wg)[jidx]
        rWg_rows = np.asarray(rWg)[jidx]
        in_maps.append({
            'xsteps': xsteps,
            'adj': bf(adj),
            'adj_s': bf(adj[:, sl]),
            'adjT_s': bf(adjT[:, sl]),
            'gcw': gcw,
            'gcwT_s': bf(gcwT[:, np.concatenate(
                [np.arange(k * N + d * NS, k * N + (d + 1) * NS)
                 for k in range(K)])]),
            'wgcT_s': np.ascontiguousarray(
                Wg_rows[:, :K * N].T.astype(ml_dtypes.bfloat16)),
            'wghT_s': np.ascontiguousarray(
                Wg_rows[:, K * N:].T.astype(ml_dtypes.float8_e4m3)),
            'rwxT_s': np.ascontiguousarray(
                rWg_rows[:, :N].T.astype(ml_dtypes.bfloat16)),
            'rwghT_s': np.ascontiguousarray(
                rWg_rows[:, N:].T.astype(ml_dtypes.float8_e4m3)),
            'bg_s': f(np.asarray(bg)[jidx]),
            'rbg_s': f(np.asarray(rbg)[jidx]),
            'nw': f(inputs['neighbor_w']),
        })
    return in_maps


def _assemble(results, inputs):
    H = np.zeros((R, N), np.float32)
    rH = np.zeros((R, N), np.float32)
    v2sum = np.zeros(2, np.float64)
    v2sq = np.zeros(2, np.float64)
    for d, res in enumerate(results):
        sl = slice(d * NS, (d + 1) * NS)
        H[:, sl] = res['H_out'].T
        rH[:, sl] = res['rH_out'].T
        v2p = res['v2part'].reshape(P, K, 4).astype(np.float64)
        v2sum += v2p[:, :, 0:2].sum((0, 1))
        v2sq += v2p[:, :, 2:4].sum((0, 1))
    v1p = results[0]['v1part'].reshape(P, NCH, 4).astype(np.float64)
    v1sum = v1p[:, :, 0:2].sum((0, 1))
    v1sq = v1p[:, :, 2:4].sum((0, 1))
    n1 = B * N
    n2 = B * K * N
    var1 = (v1sq - v1sum ** 2 / n1) / (n1 - 1)
    var2 = (v2sq - v2sum ** 2 / n2) / (n2 - 1)
    c = float(np.asarray(inputs['c'])[0])
    var1r = np.repeat(var1, B)[:, None].astype(np.float32)   # rows are c-major
    var2r = np.repeat(var2, B)[:, None].astype(np.float32)
    pred = (H * var1r * c + rH * var2r) / (var1r + var2r * c)  # (R, N)
    predB = pred.reshape(C, B, N).transpose(1, 0, 2)           # (B, C, N)
    conv_w = np.asarray(inputs['conv_w'], dtype=np.float32)    # (C*PRED, C)
    conv_b = np.asarray(inputs['conv_b'], dtype=np.float32)
    out = np.tensordot(predB, conv_w, axes=([1], [1]))         # (B, N, C*PRED)
    out = np.transpose(out, (0, 2, 1)) + conv_b[None, :, None]  # (B, C*PRED, N)
    return np.ascontiguousarray(out.reshape(B, C, N, PRED).astype(np.float32))


def kernel(**inputs):
    nc = _get_nc()
    in_maps = _marshal(inputs)
    res = run_bass_kernel_spmd(nc, in_maps, core_ids=list(range(G)))
    return _assemble(res.results, inputs)


if __name__ == "__main__":
    import time
    t0 = time.time()
    nc = _get_nc()
    print(f"build {time.time()-t0:.1f}s")
